# revision 52
# baseline (speedup 1.0000x reference)
"""Causal multi-head self-attention on 8 Trainium2 NeuronCores.

B=2, N=2048, D=1024, H=16 heads of d=64. Head-parallel sharding: core c
owns heads 2c, 2c+1. Each core reads the full (host-tiled, bf16) X and its
128-column slice of Wq/Wk/Wv (and 128-row slice of Wo), computes
Q^T/K^T/V for its 2 heads, runs causal flash-style attention entirely in
"transposed" layout, applies its Wo slice, and writes a full-shape partial
output. The host sums the 8 partials + bo.

v2 changes over the bf16 baseline:
  - Q^T/K^T are evacuated to fp8(e4m3) and repacked (partition-pair DMA)
    into [32, 2, N] so the S^T matmuls run in DoubleRow perf mode
    (2 fp8 MACs/PE-cell/cycle): S matmul cost halves.
  - The two heads' S^T blocks land in one 2-bank PSUM tile [128, 2, 512]
    and a single ACT exp instruction (scale=1/8 folds the 1/sqrt(dk))
    covers both heads: ACT instruction count halves.
  - X^T is host-tiled [128, 8k, BN] and DMA'd as 8 big slab descriptors.
  - proj/out-proj PSUM share one 2-bank pool; s pool 2x2 banks; av 2.
"""

import numpy as np

B, N, D, H, DK, DV = 2, 2048, 1024, 16, 64, 64
NCORES = 8
HPC = H // NCORES  # heads per core = 2
BN = B * N  # 4096
NQ_CHUNK = 512  # query chunk (psum free dim)
NK_BLK = 128  # key block (psum partition dim)
N_JCH = N // NQ_CHUNK  # 4 q-chunks per batch
N_KBLK = N // NK_BLK  # 16 k-blocks per batch
KT_PER_D = D // 128  # 8 contraction tiles for the projections
NBLK_ALL = BN // NK_BLK  # 32 n-blocks over both batches
NCG = 8  # X slab groups (512 cols each)
CGW = BN // NCG

_STATE = {}
DEBUG = False


def _build_nc(iters=1):
    import concourse.bacc as bacc
    import concourse.mybir as mybir
    import concourse.tile as tile
    from concourse.masks import make_upper_triangular

    f32 = mybir.dt.float32
    bf16 = mybir.dt.bfloat16
    fp8 = mybir.dt.float8e4
    AF = mybir.ActivationFunctionType
    DR = mybir.MatmulPerfMode.DoubleRow

    nc = bacc.Bacc("TRN2", target_bir_lowering=False, debug=False)

    xt_d = nc.dram_tensor("xt", [128, KT_PER_D, BN], bf16, kind="ExternalInput")
    wq_d = nc.dram_tensor("wq", [128, KT_PER_D, 128], bf16, kind="ExternalInput")
    wk_d = nc.dram_tensor("wk", [128, KT_PER_D, 128], bf16, kind="ExternalInput")
    wv_d = nc.dram_tensor("wv", [128, KT_PER_D, 128], bf16, kind="ExternalInput")
    wo_d = nc.dram_tensor("wo", [128, D], bf16, kind="ExternalInput")
    bq_d = nc.dram_tensor("bq", [128, 1], f32, kind="ExternalInput")
    bk_d = nc.dram_tensor("bk", [128, 1], f32, kind="ExternalInput")
    bv_d = nc.dram_tensor("bv", [128, 1], f32, kind="ExternalInput")
    out_d = nc.dram_tensor("out", [BN, D], bf16, kind="ExternalOutput")
    if DEBUG:
        dqk_d = nc.dram_tensor("dqk", [32, 2, 2, 512], fp8, kind="ExternalOutput")
        de_d = nc.dram_tensor("de", [128, HPC, 512], bf16, kind="ExternalOutput")
        dbc_d = nc.dram_tensor("dbc", [64, HPC, 512], f32, kind="ExternalOutput")
        davn_d = nc.dram_tensor("davn", [128, 512], bf16, kind="ExternalOutput")

    with tile.TileContext(nc) as tc:
        with (
            tc.tile_pool(name="const", bufs=1) as const,
            tc.tile_pool(name="xtp", bufs=1) as xtp,
            tc.tile_pool(name="persist", bufs=1) as persist,
            tc.tile_pool(name="avn", bufs=3) as avnp,
            tc.tile_pool(name="expp", bufs=6) as expp,
            tc.tile_pool(name="s0p", bufs=3) as s0p,
            tc.tile_pool(name="bcp", bufs=3) as bcp,
            tc.tile_pool(name="h1tp", bufs=2) as h1tp,
            tc.tile_pool(name="osb", bufs=3) as osbp,
        ):
            # ---- constants; wq + first X slab first so proj 0 starts ASAP ----
            wq_sb = const.tile([128, KT_PER_D, 128], bf16, tag="wq")
            wk_sb = const.tile([128, KT_PER_D, 128], bf16, tag="wk")
            wv_sb = const.tile([128, KT_PER_D, 128], bf16, tag="wv")
            nc.sync.dma_start(wq_sb[:], wq_d[:])
            xt_t = []
            for cg in range(NCG):
                t = xtp.tile([128, KT_PER_D, CGW], bf16, tag=f"xt{cg}", name=f"xt{cg}")
                xt_t.append(t)
            # first slab split by k so proj-0 k=0..1 can start early
            nc.sync.dma_start(xt_t[0][:, 0:2, :], xt_d[:, 0:2, 0:CGW])
            nc.sync.dma_start(xt_t[0][:, 2:KT_PER_D, :], xt_d[:, 2:KT_PER_D, 0:CGW])
            nc.sync.dma_start(wk_sb[:], wk_d[:])
            nc.sync.dma_start(wv_sb[:], wv_d[:])
            bq_sb = const.tile([128, 1], f32, tag="bq")
            bk_sb = const.tile([128, 1], f32, tag="bk")
            nc.sync.dma_start(bq_sb[:], bq_d[:])
            nc.sync.dma_start(bk_sb[:], bk_d[:])
            bv_sb = const.tile([128, 1], f32, tag="bv")
            nc.sync.dma_start(bv_sb[:], bv_d[:])
            wo_sb = const.tile([128, D], bf16, tag="wo")
            nc.sync.dma_start(wo_sb[:], wo_d[:])
            # causal keep-mask: mask[p, f] = 1.0 iff f >= p
            mask = const.tile([128, 128], bf16, tag="mask")
            make_upper_triangular(nc, mask[:], val=1.0, diag=True)

            # ---- remaining X^T slabs: one big DMA per 512-col group ----
            for cg in range(1, NCG):
                nc.sync.dma_start(
                    xt_t[cg][:], xt_d[:, :, cg * CGW : (cg + 1) * CGW]
                )

            def xt_slice(k, c0, c1):
                cg = c0 // CGW
                assert c1 <= (cg + 1) * CGW
                return xt_t[cg][:, k, c0 - cg * CGW : c1 - cg * CGW]

            # ---- persistent activations ----
            # Q^T and K^T fp8 interleaved in one tile: [:, 0, :] = Q, [:, 1, :] = K
            QKT8 = persist.tile([128, 2, BN], fp8, tag="qkt8")
            VT = persist.tile([128, BN], bf16, tag="vt")
            # packed fp8 layout for DoubleRow S: [32 pair, 2 sub, 2 q/k, BN]
            QKTp = [persist.tile([32, 2, 2, BN], fp8, tag=f"qktp{h}", name=f"qktp{h}")
                    for h in range(HPC)]
            # V blocks with interleaved ones cols: [V_h0 | 1 | V_h1 | 1]
            V_sb = persist.tile([128, NBLK_ALL, 130], bf16, tag="vsb")
            vview = V_sb.rearrange("p i (g c) -> p i g c", c=65)
            nc.vector.memset(vview[:, :, :, 64:65], 1.0)

            def body():
                # PSUM budget (8 banks): mm 2 + s 2x2 + av 2
                with (
                    tc.tile_pool(name="pp", bufs=2, space="PSUM") as pp,
                    tc.tile_pool(name="sp", bufs=2, space="PSUM") as sp,
                    tc.tile_pool(name="avp", bufs=2, space="PSUM") as avp,
                ):
                    def emit_repack(c0, c1):
                        """Partition-pair repack of interleaved Q/K fp8 into
                        [32, 2 sub, 2 q/k, *] — 4 DMAs per span."""
                        with tc.high_priority():
                            for h in range(HPC):
                                for s in range(2):
                                    p0 = 64 * h + 32 * s
                                    nc.sync.dma_start(
                                        QKTp[h][:, s, :, c0:c1],
                                        QKT8[p0 : p0 + 32, :, c0:c1],
                                    )

                    def emit_proj_chunk(j8, pump=False, which=("q", "k", "v"),
                                        repack_span=None):
                        """Q/K/V^T projection for one 512-col chunk; yields
                        after each PE instruction so it can be pumped as
                        filler inside the attention loop. After the K evac,
                        emits the Q/K repack DMAs for repack_span."""
                        c0, c1 = j8 * NQ_CHUNK, (j8 + 1) * NQ_CHUNK
                        for w_sb, b_sb, dst, nm in (
                            (wq_sb, bq_sb, QKT8[:, 0, :], "q"),
                            (wk_sb, bk_sb, QKT8[:, 1, :], "k"),
                            (wv_sb, bv_sb, VT, "v"),
                        ):
                            if nm not in which:
                                continue
                            ps = pp.tile(
                                [128, NQ_CHUNK], f32, tag="mm", name=f"ps{nm}{j8}"
                            )
                            for k in range(KT_PER_D):
                                nc.tensor.matmul(
                                    ps[:],
                                    w_sb[:, k, :],
                                    xt_slice(k, c0, c1),
                                    start=(k == 0),
                                    stop=(k == KT_PER_D - 1),
                                )
                                if pump:
                                    yield
                            nc.vector.tensor_scalar_add(dst[:, c0:c1], ps[:], b_sb[:, 0:1])
                            if nm == "k" and repack_span is not None:
                                emit_repack(*repack_span)
                            if pump:
                                yield

                    def emit_v_layout(b, q):
                        # one 512-col quarter: xbar transpose to contiguous
                        # scratch, then DVE-copy into the interleaved layout.
                        nb0 = b * N_KBLK + q * 4
                        for h in range(HPC):
                            vtmp = h1tp.tile(
                                [128, 4, 64], bf16, tag="vtmp", name=f"vtmp{b}_{q}_{h}"
                            )
                            nc.sync.dma_start_transpose(
                                vtmp[:],
                                VT[h * 64 : (h + 1) * 64, b * N + q * 512 : b * N + (q + 1) * 512],
                            )
                            nc.vector.tensor_copy(
                                V_sb[:, nb0 : nb0 + 4, 65 * h : 65 * h + 64], vtmp[:]
                            )

                    def gen_b0_tail():
                        for j8 in range(2, N_JCH):
                            span = (j8 * NQ_CHUNK, (j8 + 1) * NQ_CHUNK)
                            yield from emit_proj_chunk(j8, pump=True, repack_span=span)
                            emit_v_layout(0, j8)
                            yield

                    def gen_b1_tail():
                        for j8 in range(N_JCH, 2 * N_JCH):
                            span = None
                            if j8 in (5, 7):
                                span = ((j8 - 1) * NQ_CHUNK, (j8 + 1) * NQ_CHUNK)
                            yield from emit_proj_chunk(j8, pump=True, repack_span=span)
                            emit_v_layout(1, j8 - N_JCH)
                            yield

                    def gen_out(b, j, AVnj, tail=False):
                        """Output projection for one normalized q-chunk; one
                        coalesced 512-row output DMA per chunk."""
                        row0 = b * N + j * NQ_CHUNK
                        osb = osbp.tile(
                            [128, NQ_CHUNK // 128, D], bf16, tag="osb",
                            name=f"osb{b}_{j}",
                        )
                        for nb in range(NQ_CHUNK // 128):
                            for half in range(2):
                                with tc.high_priority(offset=-120):
                                    o_ps = pp.tile(
                                        [128, 512], f32, tag="mm",
                                        name=f"o{b}_{j}_{nb}_{half}",
                                    )
                                    nc.tensor.matmul(
                                        o_ps[:],
                                        AVnj[:, nb * 128 : (nb + 1) * 128],
                                        wo_sb[:, half * 512 : (half + 1) * 512],
                                        start=True,
                                        stop=True,
                                    )
                                    if tail:
                                        nc.scalar.activation(
                                            osb[:, nb, half * 512 : (half + 1) * 512],
                                            o_ps[:], AF.Identity,
                                        )
                                    else:
                                        nc.vector.tensor_copy(
                                            osb[:, nb, half * 512 : (half + 1) * 512],
                                            o_ps[:],
                                        )
                                yield
                        # dst rows row0 + nb*128 + p <- osb[p, nb, :]
                        dst = out_d[row0 : row0 + NQ_CHUNK, :].rearrange(
                            "(nb p) d -> p nb d", p=128
                        )
                        with tc.high_priority(offset=-120):
                            nc.sync.dma_start(dst, osb[:])
                        yield

                    pending = []

                    def pump(n):
                        done = 0
                        while pending and done < n:
                            try:
                                next(pending[0])
                                done += 1
                            except StopIteration:
                                pending.pop(0)

                    # chunks 0+1 projections + repacks, then attention starts
                    for j8 in (0, 1):
                        span = (j8 * NQ_CHUNK, (j8 + 1) * NQ_CHUNK)
                        for _ in emit_proj_chunk(j8, pump=False, repack_span=span):
                            pass
                        emit_v_layout(0, j8)
                    if DEBUG:
                        nc.sync.dma_start(dqk_d[:], QKTp[0][:, :, :, 0:512])
                    pending.append(gen_b0_tail())
                    pending.append(gen_b1_tail())

                    for b in range(B):
                        cb = b * N
                        j_order = range(N_JCH) if b == 0 else range(N_JCH - 1, -1, -1)
                        for j in j_order:
                            kmax = (j + 1) * (NQ_CHUNK // NK_BLK)
                            q0 = cb + j * NQ_CHUNK
                            av_ps = [
                                avp.tile([65, NQ_CHUNK], f32, tag="av", name=f"av{b}_{j}_{h}")
                                for h in range(HPC)
                            ]
                            for i in range(kmax):
                                r = i - j * (NQ_CHUNK // NK_BLK)
                                f0 = 128 * r if r > 0 else 0
                                s_ps = sp.tile(
                                    [128, HPC, NQ_CHUNK], f32, tag="s",
                                    name=f"s{b}_{j}_{i}",
                                )
                                for h in range(HPC):
                                    nc.tensor.matmul(
                                        s_ps[:, h, f0:NQ_CHUNK],
                                        QKTp[h][:, :, 1, cb + i * 128 : cb + (i + 1) * 128],
                                        QKTp[h][:, :, 0, q0 + f0 : q0 + NQ_CHUNK],
                                        start=True,
                                        stop=True,
                                        perf_mode=DR,
                                    )
                                pump(2)
                                et = expp.tile(
                                    [128, HPC, NQ_CHUNK], bf16, tag="exp",
                                    name=f"e{b}_{j}_{i}",
                                )
                                nc.scalar.activation(
                                    et[:, :, f0:NQ_CHUNK], s_ps[:, :, f0:NQ_CHUNK],
                                    AF.Exp, scale=0.125,
                                )
                                if r >= 0:
                                    for h in range(HPC):
                                        nc.vector.tensor_mul(
                                            et[:, h, f0 : f0 + 128],
                                            et[:, h, f0 : f0 + 128],
                                            mask[:],
                                        )
                                if DEBUG and b == 0 and j == 0 and i == 0:
                                    nc.sync.dma_start(de_d[:], et[:, :, 0:512])
                                pump(2)
                                for h in range(HPC):
                                    nc.tensor.matmul(
                                        av_ps[h][:, f0:NQ_CHUNK],
                                        V_sb[:, b * N_KBLK + i, 65 * h : 65 * (h + 1)],
                                        et[:, h, f0:NQ_CHUNK],
                                        start=(i == 0),
                                        stop=(i == kmax - 1),
                                        skip_group_check=True,
                                    )
                            # normalize this chunk straight out of PSUM
                            AVnj = avnp.tile(
                                [128, NQ_CHUNK], bf16, tag="avn", name=f"avn{b}_{j}"
                            )
                            rc = s0p.tile(
                                [65, HPC, NQ_CHUNK], f32, tag="rc", name=f"rc{b}_{j}"
                            )
                            for h in range(HPC):
                                nc.vector.reciprocal(
                                    rc[64:65, h, :], av_ps[h][64:65, :]
                                )
                            # gpsimd broadcast reads partition 0 on HW: DMA-hop
                            # the recip row down from partition 64 first.
                            s0 = s0p.tile(
                                [1, HPC, NQ_CHUNK], f32, tag="s0", name=f"s0_{b}_{j}"
                            )
                            nc.sync.dma_start(s0[0:1, :, :], rc[64:65, :, :])
                            bc = bcp.tile(
                                [64, HPC, NQ_CHUNK], f32, tag="bc", name=f"bc{b}_{j}"
                            )
                            nc.gpsimd.partition_broadcast(bc[:], s0[0:1, :, :])
                            for h in range(HPC):
                                if h == 0:
                                    nc.vector.tensor_mul(
                                        AVnj[0:64, :], av_ps[0][0:64, :], bc[:, 0, :]
                                    )
                                else:
                                    h1t = h1tp.tile(
                                        [64, NQ_CHUNK], bf16, tag="h1t", name=f"h1t{b}_{j}"
                                    )
                                    nc.vector.tensor_mul(
                                        h1t[:], av_ps[1][0:64, :], bc[:, 1, :]
                                    )
                                    nc.sync.dma_start(AVnj[64:128, :], h1t[:])
                            if DEBUG and b == 0 and j == 0:
                                nc.sync.dma_start(dbc_d[:], bc[:])
                                nc.sync.dma_start(davn_d[:], AVnj[:])
                            pending.append(
                                gen_out(b, j, AVnj, tail=(b == 1 and j == 0))
                            )
                        if b == 0:
                            # batch-1 projections/repacks must finish before
                            # its attention
                            pump(10**9)
                    pump(10**9)

            if iters > 1:
                with tc.For_i(0, iters, 1):
                    body()
            else:
                body()

    nc.compile()
    return nc


def _prep_in_maps(X, Wq, bq, Wk, bk, Wv, bv, Wo, bo):
    import ml_dtypes

    bf16 = ml_dtypes.bfloat16

    def _pkm(w):  # [D, 128] -> [128 partition, k, 128] tile layout
        return np.ascontiguousarray(
            w.reshape(KT_PER_D, 128, 128).transpose(1, 0, 2)
        ).astype(bf16)

    Xf = np.asarray(X, dtype=np.float32).reshape(BN, D)
    # host-tiled X^T: xt[p, k, n] = X[n, k*128 + p]
    xt = np.ascontiguousarray(
        Xf.T.reshape(KT_PER_D, 128, BN).transpose(1, 0, 2)
    ).astype(bf16)
    in_maps = []
    for c in range(NCORES):
        s = slice(c * 128, (c + 1) * 128)
        in_maps.append(
            {
                "xt": xt,
                "wq": _pkm(np.asarray(Wq, np.float32)[:, s]),
                "wk": _pkm(np.asarray(Wk, np.float32)[:, s]),
                "wv": _pkm(np.asarray(Wv, np.float32)[:, s]),
                "wo": np.ascontiguousarray(np.asarray(Wo, np.float32)[s, :]).astype(bf16),
                "bq": np.ascontiguousarray(np.asarray(bq, np.float32)[s].reshape(128, 1)),
                "bk": np.ascontiguousarray(np.asarray(bk, np.float32)[s].reshape(128, 1)),
                "bv": np.ascontiguousarray(np.asarray(bv, np.float32)[s].reshape(128, 1)),
            }
        )
    return in_maps


def _get_nc(iters=1):
    key = ("nc", iters)
    if key not in _STATE:
        _STATE[key] = _build_nc(iters)
    return _STATE[key]


def kernel(**inputs) -> np.ndarray:
    from concourse import bass_utils

    nc = _get_nc()
    in_maps = _prep_in_maps(**inputs)
    res = bass_utils.run_bass_kernel_spmd(nc, in_maps, core_ids=list(range(NCORES)))
    acc = np.zeros((BN, D), dtype=np.float32)
    for r in res.results:
        acc += np.asarray(r["out"], dtype=np.float32)
    acc += np.asarray(inputs["bo"], np.float32)[None, :]
    return acc.reshape(B, N, D)
